# revision 105
# baseline (speedup 1.0000x reference)
"""Trainium2 Bass kernel for nn_BottlenectedAttention.

Algorithmic reduction (unchanged from baseline): the reference consumes only
rows m=0 and m=1029 of the attention output, so per batch the whole attention
collapses to 20 logit columns ((ms, h) pairs) against an effective query
matrix  wq_eff[b] = Wq_h @ k_sel[b] / sqrt(DK)  of shape [E, 20], followed by
ctx[b, pair, :] = softmax_n(logits) @ feats[b]  and O(1)-sized host math.

Device schedule (final):
* everything on the input path is fp8e4m3 (feats in both layouts, wq_eff,
  identity): 1.13 MB/core (vs 2.26 MB bf16 dual-layout). wq_eff values
  (~0.01) sit in e4m3's subnormal range, so the host scales them by 64 and
  the exp activation compensates with scale=1/64.
* DMA bandwidth is descriptor-size bound (~2x throughput at 4KB/partition
  lines vs 2KB), so the input ships as TWO whole-tensor DMAs on the sync
  HWDGE ring: [wq const | identity | feats^T] (4.7KB lines — the const
  prefix rides along because the ACT ring has a ~2.2us first-byte lag that
  otherwise gates the logits), then natural feats (4.1KB lines). Each is
  split [bulk | 256B tail] with the completion inc on the tail: per-engine
  FIFO puts the tail desc after the bulk descs, and its tiny WAW receipt
  fires the semaphore right after the chunk's last byte (a bulk-attached
  inc pays a 1-3us receipt penalty; splitting the inc between
  bulk+sentinel semaphores is RACY since 8 fast engines can reach the
  count while slow engines still have bytes in flight).
* compute pipelined over the two 128-column halves (nb): logits(nb) ->
  exp(nb) -> PE-transpose(p_nb) -> DVE cast to fp8 -> ctx(nb). DoubleRow
  would halve the logits but is incompatible with PE column-grouping.
* exps are issued WITHOUT accum_out so the p^T transposes start
  immediately; throwaway exps recompute the softmax sums off the critical
  path.
* each 513-col natural-feats slab carries a trailing ones-column, so the
  ctx matmul accumulates the softmax denominator as PSUM column 256 of the
  second bank for free (no accumulator reads or sum copies on the exit
  path).
* ctx accumulates into two PSUM banks (e<256 / e>=256+s) so the final
  f32 -> bf16 output cast runs split across DVE and ACT (Identity shares
  the Exp act table => no reload) in parallel on different banks; the
  whole output then ships as ONE sync-ring DMA — each engine's measured
  tail is its own serial semaphore-reset chain, which starts only after
  its bk_end, so keeping Scalar's exit path short matters more than
  parallel output DMAs.

Sharding: sequence dim, 256 rows per core x 8 cores; rows [2048, 2054) are a
host-side 9th flash shard.
"""
import sys

sys.path.insert(0, "/opt/trn_rl_repo")

import numpy as np

import concourse.bass as bass
import concourse.bacc as bacc
from concourse import mybir
from concourse.bass_utils import run_bass_kernel_spmd

E, HID, NH, DK, BTNK = 512, 640, 10, 64, 4
B, LA, LV = 4, 1024, 1024
L = LA + 1 + BTNK + LV + 1          # 2054
NPAIR = 2 * NH                       # 20 (ms, h) pairs per batch
MPAD = 32                            # col-group stride (batch b -> cols 32b..)
NCORES = 8
NSL = 256                            # per-core slice width
NKT = E // 128                       # 4 k-tiles over the embedding dim
NNB = NSL // 128                     # 2 n-halves of 128
WARM_MM = 8                          # HAM warmup matmuls during input DMA
WQSCALE = 64.0                       # host premultiplies wq_eff (fp8 range)
SLICES = [(c * NSL, c * NSL + NSL) for c in range(NCORES)]

F32 = mybir.dt.float32
BF16 = mybir.dt.bfloat16
F8 = mybir.dt.float8e4

FT_H = NKT * B * 128                 # 2048 cols per nb-half of feats^T
FN_H = B * (E + 1)                   # 2052 cols per nb-half of natural feats
                                     # (col 512 of each 513-col slab is 1.0:
                                     # the ctx matmul then yields the softmax
                                     # denominator as output column 256 of
                                     # the second PSUM bank for free)


def _pos_encoding(Ln, d):
    pos = np.arange(Ln, dtype=np.float32)[:, None]
    div = np.exp(np.arange(0, d, 2, dtype=np.float32) * (-np.log(10000.0) / d))
    pe = np.zeros((Ln, d), dtype=np.float32)
    pe[:, 0::2] = np.sin(pos * div).astype(np.float32)
    pe[:, 1::2] = np.cos(pos * div).astype(np.float32)
    return pe


def build_program():
    nc = bacc.Bacc()

    # wq const + identity ride at the FRONT of the feats^T tensor: one
    # 4.7KB/partition-line DMA on the sync ring covers everything the
    # logits need (the ACT ring has a ~2.2us first-byte lag, so a separate
    # const DMA there was gating the whole pipeline)
    ft_d = nc.declare_dram_parameter("ft", [128, 640 + 2 * FT_H], F8,
                                     isOutput=False)
    fn_d = nc.declare_dram_parameter("fn", [128, 2 * FN_H], F8, isOutput=False)
    # only partitions 0..115 carry data (batch b's 20 pairs live at rows
    # 32b..32b+19), so ship 116 rows instead of 128
    out_d = nc.declare_dram_parameter("octx", [116, E + 4], BF16, isOutput=True)

    from contextlib import ExitStack
    with ExitStack() as st:
        ec = st.enter_context
        # SBUF
        ftc_s = ec(nc.sbuf_tensor("ftc_s", [128, 640 + 2 * FT_H], F8))
        fN_s = ec(nc.sbuf_tensor("fN_s", [128, 2 * FN_H], F8))
        idb_s = ec(nc.sbuf_tensor("idb_s", [128, 128], BF16))
        p_s = ec(nc.sbuf_tensor("p_s", [128, NSL], BF16))
        pT_s = ec(nc.sbuf_tensor("pT_s", [128, NSL], F8))
        dum2_s = ec(nc.sbuf_tensor("dum2_s", [128, 128], BF16))
        s_s = ec(nc.sbuf_tensor("s_s", [128, 2], F32))
        dum_s = ec(nc.sbuf_tensor("dum_s", [128, 1], F32))
        octx_s = ec(nc.sbuf_tensor("octx_s", [128, E + 4], BF16))
        warm_s = ec(nc.sbuf_tensor("warm_s", [128, 256], BF16))
        # PSUM (bank granular)
        ps_w = ec(nc.psum_tensor("ps_w", [128, 256], F32))    # warmup sink
        ps_l0 = ec(nc.psum_tensor("ps_l0", [128, 128], F32))  # logits nb0
        ps_l1 = ec(nc.psum_tensor("ps_l1", [128, 128], F32))  # logits nb1
        ps_t0 = ec(nc.psum_tensor("ps_t0", [128, 128], BF16))  # p^T nb0
        ps_t1 = ec(nc.psum_tensor("ps_t1", [128, 128], BF16))  # p^T nb1
        ps_c0 = ec(nc.psum_tensor("ps_c0", [128, 256], F32))  # ctx e<256
        ps_c1 = ec(nc.psum_tensor("ps_c1", [128, 257], F32))  # ctx e>=256 + s
        # semaphores
        semA = ec(nc.semaphore("semA"))    # feats^T landed
        semB0 = ec(nc.semaphore("semB0"))  # natural feats nb0 landed
        semB1 = ec(nc.semaphore("semB1"))  # natural feats nb1 landed
        semC = ec(nc.semaphore("semC"))    # wq const + identity landed
        ssem = ec(nc.semaphore("ssem"))    # softmax sums copied into octx
        oasem = ec(nc.semaphore("oasem"))  # ACT output cast datapath done
        junk = ec(nc.semaphore("junk"))    # bulk DMA incs nobody waits on
        lsem = ec(nc.semaphore("lsem"))    # logits nb0/nb1 done
        esem = ec(nc.semaphore("esem"))    # exp nb0/nb1/nb1-sum done
        tsem = ec(nc.semaphore("tsem"))    # transposes done
        dsem = ec(nc.semaphore("dsem"))    # pT casts done
        xsem = ec(nc.semaphore("xsem"))    # ctx bank0/bank1 done
        csem = ec(nc.semaphore("csem"))    # output casts done
        isem = ec(nc.semaphore("isem"))    # bf16 identity ready

        ident = ftc_s[:, E:E + 128]

        def wq1(et, b):
            # [128, 32]: one k-tile of the effective query weights
            o = et * 128 + MPAD * b
            return ftc_s[:, o:o + MPAD]

        def ftT1(nb, et, b):
            # [128, 128]: matching k-tile of feats^T
            o = 640 + nb * FT_H + (et * B + b) * 128
            return ftc_s[:, o:o + 128]

        def fN(nb, b, eh):
            o = nb * FN_H + b * (E + 1) + eh * 256
            return fN_s[:, o:o + (256 if eh == 0 else 257)]

        # --- pre-Block (main-block) instructions: these execute on each
        # engine right after its framework preamble, BEFORE the bk entry
        # handshake — the input stream, PE warmup, and act-table load all
        # start ~1us earlier than they would inside the Block.
        # feats^T first and ALONE on the engines (fN queued behind it on the
        # same ring): 4KB/partition descriptors at full fan-out. Each chunk
        # is split [bulk | small tail]; the tail carries the completion inc:
        # per-engine FIFO means every engine's tail desc runs after its bulk
        # descs, and the tail's tiny WAW receipt fires the semaphore right
        # after the chunk's last byte.
        TL = 64
        FTC = 640 + 2 * FT_H
        nc.sync.dma_start(out=ftc_s[:, :FTC - TL],
                          in_=ft_d[:, :FTC - TL]).then_inc(junk, 16)
        nc.sync.dma_start(out=ftc_s[:, FTC - TL:],
                          in_=ft_d[:, FTC - TL:]).then_inc(semA, 16)
        # natural feats as ONE 4KB-line DMA (nb-split chunks would degrade
        # to ~1.8KB descriptors and run ~30% slower)
        nc.sync.dma_start(out=fN_s[:, :2 * FN_H - TL],
                          in_=fn_d[:, :2 * FN_H - TL]).then_inc(junk, 16)
        nc.sync.dma_start(out=fN_s[:, 2 * FN_H - TL:],
                          in_=fn_d[:, 2 * FN_H - TL:]).then_inc(semB1, 16)
        # HAM warmup while inputs stream. warm_s read uninitialized on
        # purpose; ps_w never consumed.
        for _ in range(WARM_MM):
            nc.tensor.matmul(ps_w[:, :], warm_s[:, :128], warm_s[:, :256],
                             start=True, stop=True)
        # dummy activation so the act-table load happens early
        nc.scalar.activation(out=dum_s[:1, :], in_=dum_s[:1, :],
                             func=mybir.ActivationFunctionType.Exp,
                             bias=0.0, scale=0.0)

        with nc.Block("bk") as block:

            @block.sync
            def _(sync):
                # whole output on the sync ring: shipping half from the ACT
                # ring delays Scalar's bk_end, and each engine's fixed
                # semaphore-reset chain (the measured window's tail) starts
                # only after its own bk_end
                sync.wait_ge(csem, 2)
                sync.dma_start(out=out_d[:],
                               in_=octx_s[0:116, :]).then_inc(junk, 16)
                # no completion wait: bk_end's SP Drain waits for the ring

            @block.gpsimd
            def _(gpsimd):
                # no gpsimd work (cannot touch PSUM; Pool DMA is slow SWDGE);
                # empty body still routes Pool to bk_end for the exit barrier.
                pass

            @block.tensor
            def _(tensor):
                # logits: batches in col-groups, accumulate over k-tiles
                # (DoubleRow would halve this but is incompatible with PE
                # column-grouping — fails the walrus ISA check)
                tensor.wait_ge(semA, 16)
                for nb, psl in ((0, ps_l0), (1, ps_l1)):
                    for et in range(NKT):
                        for b in range(B):
                            mm = tensor.matmul(
                                psl[32 * b:32 * b + MPAD, :],
                                wq1(et, b),
                                ftT1(nb, et, b),
                                start=(et == 0), stop=(et == NKT - 1),
                                tile_position=(0, 32 * b),
                                skip_group_check=True,
                            )
                    mm.then_inc(lsem, 1)
                # p^T nb0 then nb1
                tensor.wait_ge(isem, 1)
                tensor.wait_ge(esem, 1)
                tensor.transpose(ps_t0[:, :], p_s[:, 0:128],
                                 idb_s[:, :]).then_inc(tsem, 1)
                tensor.wait_ge(esem, 2)
                tensor.transpose(ps_t1[:, :], p_s[:, 128:256],
                                 idb_s[:, :]).then_inc(tsem, 1)
                # ctx nb0 (start) then nb1 (stop); batches in col-groups;
                # e halves into separate PSUM banks for the parallel casts
                tensor.wait_ge(dsem, 1)
                tensor.wait_ge(semB1, 16)
                for b in range(B):
                    for eh, psc in ((0, ps_c0), (1, ps_c1)):
                        tensor.matmul(
                            psc[32 * b:32 * b + MPAD, :],
                            pT_s[:, 32 * b:32 * b + MPAD],
                            fN(0, b, eh),
                            start=True, stop=False,
                            tile_position=(0, 32 * b),
                            skip_group_check=True,
                        )
                tensor.wait_ge(dsem, 2)
                tensor.wait_ge(semB1, 16)
                last = {}
                for b in range(B):
                    for eh, psc in ((0, ps_c0), (1, ps_c1)):
                        last[eh] = tensor.matmul(
                            psc[32 * b:32 * b + MPAD, :],
                            pT_s[:, 128 + 32 * b:128 + 32 * b + MPAD],
                            fN(1, b, eh),
                            start=False, stop=True,
                            tile_position=(0, 32 * b),
                            skip_group_check=True,
                        )
                last[0].then_inc(xsem, 1)
                last[1].then_inc(xsem, 1)

            @block.scalar
            def _(scalar):
                # both exps WITHOUT accumulator so the p^T transposes launch
                # immediately; the row sums are recomputed into a scratch
                scalar.wait_ge(lsem, 1)
                scalar.activation(out=p_s[:, 0:128], in_=ps_l0[:, :],
                                  func=mybir.ActivationFunctionType.Exp,
                                  bias=0.0, scale=1.0 / WQSCALE).then_inc(esem, 1)
                scalar.wait_ge(lsem, 2)
                scalar.activation(out=p_s[:, 128:256], in_=ps_l1[:, :],
                                  func=mybir.ActivationFunctionType.Exp,
                                  bias=0.0, scale=1.0 / WQSCALE).then_inc(esem, 1)
                # second half of the output cast (Identity shares the Exp
                # act table -> no table reload), from its own PSUM bank;
                # column 256 of ps_c1 is the softmax denominator accumulated
                # by the ctx matmuls against the ones-column of fN
                scalar.wait_ge(xsem, 2)
                scalar.activation(out=octx_s[:, 256:513], in_=ps_c1[:, :],
                                  func=mybir.ActivationFunctionType.Identity,
                                  bias=0.0, scale=1.0).then_inc(csem, 1)

            @block.vector
            def _(vector):
                # upcast the fp8 identity to bf16 for the p^T transposes
                vector.wait_ge(semA, 16)
                vector.tensor_copy(out=idb_s[:, :],
                                   in_=ident).then_inc(isem, 1)
                vector.wait_ge(tsem, 1)
                vector.tensor_copy(out=pT_s[:, 0:128],
                                   in_=ps_t0[:, :]).then_inc(dsem, 1)
                vector.wait_ge(tsem, 2)
                vector.tensor_copy(out=pT_s[:, 128:256],
                                   in_=ps_t1[:, :]).then_inc(dsem, 1)
                vector.wait_ge(xsem, 1)
                vector.tensor_copy(out=octx_s[:, 0:256],
                                   in_=ps_c0[:, :]).then_inc(csem, 1)

    nc.finalize()
    return nc


def _install_ntff_hook():
    """The agent image's antenv lacks axon_hooks; recreate it and register the
    ctypes NTFF profile hook against the injected libaxon_pjrt.so so that
    run_bass_kernel_spmd(trace=True) can capture HW exec times."""
    import contextlib
    import ctypes
    import types

    if "antenv.axon_hooks" in sys.modules:
        return
    mod = types.ModuleType("antenv.axon_hooks")
    state = {"hook": None}
    mod.set_axon_ntff_profile_hook = lambda h: state.__setitem__("hook", h)
    mod.get_axon_ntff_profile_hook = lambda: state["hook"]
    sys.modules["antenv.axon_hooks"] = mod
    try:
        import antenv

        antenv.axon_hooks = mod
    except ImportError:
        pass

    so_path = "/opt/axon/libaxon_pjrt.so"
    try:
        lib = ctypes.CDLL(so_path)
    except OSError:
        return
    if not hasattr(lib, "axon_start_nrt_profile"):
        return
    lib.axon_start_nrt_profile.argtypes = [
        ctypes.POINTER(ctypes.c_int64),
        ctypes.c_size_t,
    ]
    lib.axon_start_nrt_profile.restype = ctypes.c_int64
    lib.axon_stop_nrt_profile.argtypes = [ctypes.c_char_p]
    lib.axon_stop_nrt_profile.restype = ctypes.c_int64

    @contextlib.contextmanager
    def _hook(output_dir, device_ids):
        import jax

        jax.devices()
        if device_ids:
            ids = (ctypes.c_int64 * len(device_ids))(*device_ids)
            rc = lib.axon_start_nrt_profile(ids, len(device_ids))
        else:
            rc = lib.axon_start_nrt_profile(None, 0)
        if rc != 0:
            raise RuntimeError(f"axon_start_nrt_profile rc={rc}")
        try:
            yield
        finally:
            n = lib.axon_stop_nrt_profile(str(output_dir).encode())
            print(f"profile: {n} file(s) written to {output_dir}", file=sys.stderr)

    state["hook"] = _hook


_CACHE = {}


def _get_program():
    if "raw" not in _CACHE:
        _CACHE["raw"] = build_program()
    return _CACHE["raw"]


def _prepare_host(inputs):
    import ml_dtypes

    f8 = ml_dtypes.float8_e4m3
    af = np.ascontiguousarray(np.asarray(inputs["audio_feat"], dtype=np.float32))
    vf = np.ascontiguousarray(np.asarray(inputs["video_feat"], dtype=np.float32))
    at = np.asarray(inputs["audio_tok"], dtype=np.float32)
    vt = np.asarray(inputs["video_tok"], dtype=np.float32)
    bt = np.asarray(inputs["btnk_toks"], dtype=np.float32)
    Wk = np.asarray(inputs["Wk"], dtype=np.float32)
    bk = np.asarray(inputs["bk"], dtype=np.float32)
    Wq = np.asarray(inputs["Wq"], dtype=np.float32)

    pe = _pos_encoding(L, E)

    raw = np.empty((B, L, E), np.float32)
    raw[:, :LA] = af
    raw[:, LA] = at[0, 0]
    raw[:, LA + 1:LA + 1 + BTNK] = bt[0]
    raw[:, LA + 1 + BTNK:LA + 1 + BTNK + LV] = vf
    raw[:, L - 1] = vt[0, 0]

    feats8 = (raw + pe[None]).astype(f8)                     # [B, L, E] fp8
    feats8f = feats8.astype(np.float64)

    # effective query vectors (f64 host math)
    f_rows = np.stack([raw[:, 0] + pe[0], raw[:, LA + 1 + BTNK] + pe[LA + 1 + BTNK]],
                      axis=1).astype(np.float64)             # [B,2,E]
    k_sel = (f_rows @ Wk.astype(np.float64) + bk).reshape(B, 2, NH, DK)
    Wq_h = Wq.astype(np.float64).reshape(E, NH, DK)
    wq_eff = np.einsum("dhx,bmhx->bdmh", Wq_h, k_sel).reshape(B, E, NPAIR)
    wq_eff = wq_eff / np.sqrt(DK)                            # [B,E,20] f64

    # const prefix: wq (scaled into fp8 normal range) | identity
    wq_pad = np.zeros((B, E, MPAD), np.float32)
    wq_pad[:, :, :NPAIR] = (wq_eff * WQSCALE).astype(np.float32)
    c0 = np.zeros((128, E + 128), np.float32)
    c0[:, :E] = (
        wq_pad.reshape(B, NKT, 128, MPAD)
        .transpose(2, 1, 0, 3)
        .reshape(128, E)
    )
    c0[:, E:E + 128] = np.eye(128, dtype=np.float32)
    c0 = c0.astype(f8)

    in_maps = []
    for c, (n0, n1) in enumerate(SLICES):
        block = feats8[:, n0:n1, :]                          # [B,256,E] fp8
        # ftT: [p=e%128, nb*2048 + et*512 + b*128 + j]
        ftT = (block.reshape(B, NNB, 128, NKT, 128)          # [b, nb, j, et, p]
               .transpose(4, 1, 3, 0, 2)                     # [p, nb, et, b, j]
               .reshape(128, NNB * FT_H))
        # fN: [p=j, nb*2052 + b*513 + e], col 512 of each slab = 1.0 (the
        # ctx matmul accumulates the softmax denominator against it)
        fpad = np.ones((B, NNB, 128, E + 1), f8)
        fpad[..., :E] = block.reshape(B, NNB, 128, E)
        fNl = (fpad.transpose(2, 1, 0, 3)                    # [p, nb, b, e+1]
               .reshape(128, NNB * FN_H))
        in_maps.append({
            "ft": np.ascontiguousarray(np.concatenate([c0, ftT], axis=1)),
            "fn": np.ascontiguousarray(fNl),
        })

    # host 9th flash shard for rows [2048, L) — same fp8 feats as the device
    n0 = NCORES * NSL
    tail = feats8f[:, n0:L]                                  # [B,6,E]
    tail_logits = np.einsum("bnd,bdp->bnp", tail, wq_eff)
    m9 = tail_logits.max(axis=1)                             # [B,20]
    p9 = np.exp(tail_logits - m9[:, None, :])
    s9 = p9.sum(axis=1)                                      # [B,20]
    ctx9 = np.einsum("bnp,bnd->bpd", p9, tail)               # [B,20,E]
    return in_maps, (m9, s9, ctx9)


def _finalize(inputs, ctxs, stats, tail_partial):
    """ctxs: [8,B,20,E] unnormalized local contexts; stats: [8,B,20,2] (m, s);
    tail_partial: host-computed 9th shard for rows [2048, 2054)."""
    Wv = np.asarray(inputs["Wv"], dtype=np.float64)
    bv = np.asarray(inputs["bv"], dtype=np.float64)
    ln_g = np.asarray(inputs["ln_g"], dtype=np.float64)
    ln_b = np.asarray(inputs["ln_b"], dtype=np.float64)
    Wap = np.asarray(inputs["Wap"], dtype=np.float64)
    bap = np.asarray(inputs["bap"], dtype=np.float64)
    Wvp = np.asarray(inputs["Wvp"], dtype=np.float64)
    bvp = np.asarray(inputs["bvp"], dtype=np.float64)

    m9, s9, ctx9 = tail_partial
    m = np.concatenate([stats[..., 0].astype(np.float64), m9[None]])   # [9,B,20]
    s = np.concatenate([stats[..., 1].astype(np.float64), s9[None]])
    ctxs = np.concatenate([ctxs.astype(np.float64), ctx9[None]])       # [9,B,20,E]
    Mg = m.max(axis=0)                                   # [B,20]
    w = np.exp(m - Mg[None])
    denom = (w * s).sum(axis=0)                          # [B,20]
    ctx_full = (w[..., None] * ctxs).sum(axis=0) / denom[..., None]

    Wv_h = Wv.reshape(E, NH, DK)
    out = np.empty((B, 2, HID), np.float64)
    for ms in range(2):
        for h in range(NH):
            out[:, ms, h * DK:(h + 1) * DK] = np.einsum(
                "bd,dx->bx", ctx_full[:, ms * NH + h], Wv_h[:, h])
    out = out + bv

    mu = out.mean(-1, keepdims=True)
    var = out.var(-1, keepdims=True)
    out_ln = (out - mu) / np.sqrt(var + 1e-5) * ln_g + ln_b

    aud = out_ln[:, 0] @ Wap + bap
    vid = out_ln[:, 1] @ Wvp + bvp
    return (((aud + vid) / 2).astype(np.float32))


def run(inputs, trace=False, mode="bf16"):
    nc = _get_program()
    in_maps, tail_partial = _prepare_host(inputs)
    kw = {}
    if trace:
        _install_ntff_hook()
        import concourse.bass_utils as bu

        bu.upload_artifacts = lambda tmpdir: str(tmpdir)
        kw = dict(trace=True, trace_cores=list(range(NCORES)))
    res = run_bass_kernel_spmd(nc, in_maps, list(range(NCORES)), **kw)
    allout = np.stack([np.asarray(r["octx"], dtype=np.float64)
                       for r in res.results])                # [8,128,E+4]
    # row 32b+pair holds batch b, pair
    rows = (MPAD * np.arange(B)[:, None] + np.arange(NPAIR)[None, :])  # [B,20]
    ctxs = allout[:, rows, :E]                               # [8,B,20,E]
    stats = np.zeros((NCORES, B, NPAIR, 2), np.float64)
    stats[..., 1] = allout[:, rows, E]
    out = _finalize(inputs, ctxs, stats, tail_partial)
    return out, res


def kernel(**inputs) -> np.ndarray:
    out, _ = run(inputs, trace=False)
    return out
